# revision 1
# baseline (speedup 1.0000x reference)
"""AudioDALLE forward (4-layer sparse-attention transformer + vocab loss).

Strategy: the dense matmuls (QKV / Wo / FFN W1,W2 / 8192-vocab logits --
~96% of model FLOPs) run on 8 Trainium2 NeuronCores via a generic Bass
SPMD matmul kernel, sharded data-parallel over batch (4) x tensor-parallel
over output columns (2).  Sequence mixing (softmax, LN, axial/conv
attention glue) runs on host fp32.  If the device path is unavailable the
kernel falls back to numpy matmuls so the output is always correct.
"""

import os
import sys
import numpy as np

# ---------------------------------------------------------------- constants
B = 4
AUDIO_LEN = 128
FEAT = 128
DIM = 512
HEADS = 8
DH = DIM // HEADS
LAYERS = 4
G = 32
IMG_LEN = G * G
VOCAB = 8192
TEXT_LEN = AUDIO_LEN + 1
KS = 5
PAD = KS // 2
PG = G + 2 * PAD
SCALE = DH ** -0.5
NEG = np.float32(-1e30)
TOTAL = AUDIO_LEN + IMG_LEN          # 1152
NPAD = TOTAL + 1                     # 1153 (dalle pads seq by 1)

ATTN_TYPES = ['axial_col' if (i - 2) % 4 == 0 else 'axial_row'
              for i in range(LAYERS - 1)] + ['conv_like']


def _conv_indices():
    rr = np.arange(G)
    q_r = np.repeat(rr, G); q_c = np.tile(rr, G)
    dr, dc = np.meshgrid(np.arange(KS), np.arange(KS), indexing='ij')
    dr = dr.reshape(-1); dc = dc.reshape(-1)
    pr = q_r[:, None] + dr[None, :]
    pc = q_c[:, None] + dc[None, :]
    pidx = pr * PG + pc
    orr = pr - PAD; occ = pc - PAD
    inb = (orr >= 0) & (orr < G) & (occ >= 0) & (occ < G)
    kidx = np.where(inb, orr * G + occ, IMG_LEN)
    causal = np.arange(G * G)[:, None] < kidx
    return pidx, causal


PIDX, CONV_MASK = _conv_indices()

# ---------------------------------------------------------------- device mm
_DEV = {"ok": None, "cache": {}}


def _device_init():
    if _DEV["ok"] is not None:
        return _DEV["ok"]
    try:
        sys.path.insert(0, '/opt/trn_rl_repo')
        import jax  # noqa
        import concourse.bass as bass  # noqa
        import concourse.mybir as mybir
        import concourse.tile as tile
        from concourse import bacc
        from concourse.bass_utils import run_bass_kernel_spmd
        _DEV["mybir"] = mybir
        _DEV["tile"] = tile
        _DEV["bacc"] = bacc
        _DEV["run"] = run_bass_kernel_spmd
        _DEV["ok"] = True
    except Exception as e:  # pragma: no cover
        print("kernel.py: device init failed, numpy fallback:", e)
        _DEV["ok"] = False
    return _DEV["ok"]


def _build_mm(k, m, n):
    """Bass module computing Y[m,n] = W[k,m].T @ AT[k,n] on each core (fp32)."""
    mybir = _DEV["mybir"]; tile = _DEV["tile"]; bacc = _DEV["bacc"]
    f32 = mybir.dt.float32
    P = 128
    kt, MT, NT = k // P, m // P, n // 512
    nc = bacc.Bacc("TRN2", target_bir_lowering=False, debug=False,
                   num_devices=8)
    W = nc.declare_dram_parameter("w", [k, m], f32, isOutput=False)
    A = nc.declare_dram_parameter("a", [k, n], f32, isOutput=False)
    Y = nc.declare_dram_parameter("y", [m, n], f32, isOutput=True)
    Wr = W.rearrange("(ko p) m -> p ko m", p=P)
    Ar = A.rearrange("(ko p) n -> p ko n", p=P)
    with tile.TileContext(nc) as tc:
        with tc.tile_pool(name="wp", bufs=2) as wp, \
             tc.tile_pool(name="apool", bufs=max(2, NT)) as apool, \
             tc.tile_pool(name="pp", bufs=2, space="PSUM") as pp, \
             tc.tile_pool(name="op", bufs=3) as op:
            a_tiles = {}
            for mi in range(MT):
                wt = wp.tile([P, kt, P], f32, tag="wt")
                nc.sync.dma_start(wt[:], Wr[:, :, mi * P:(mi + 1) * P])
                for ni in range(NT):
                    if ni not in a_tiles:
                        at = apool.tile([P, kt, 512], f32, tag="at")
                        nc.sync.dma_start(
                            at[:], Ar[:, :, ni * 512:(ni + 1) * 512])
                        a_tiles[ni] = at
                    at = a_tiles[ni]
                    ps = pp.tile([P, 512], f32)
                    for ko in range(kt):
                        nc.tensor.matmul(ps[:], wt[:, ko, :], at[:, ko, :],
                                         start=(ko == 0), stop=(ko == kt - 1))
                    ot = op.tile([P, 512], f32)
                    nc.any.tensor_copy(ot[:], ps[:])
                    nc.sync.dma_start(
                        Y[mi * P:(mi + 1) * P, ni * 512:(ni + 1) * 512], ot[:])
    nc.finalize()
    return nc


def _pad_to(x, r, c):
    out = np.zeros((r, c), np.float32)
    out[:x.shape[0], :x.shape[1]] = x
    return out


def batched_mm(A_list, Wfull):
    """Y_list[b] = A_list[b] @ Wfull   for b in 0..3.

    On device: core c in 0..3 computes batch c with W[:, :m/2];
    core c+4 computes batch c with W[:, m/2:].  Host concatenates.
    Falls back to numpy if the device path is unavailable.
    """
    if not _device_init():
        return [a.astype(np.float32) @ Wfull.astype(np.float32)
                for a in A_list]
    kdim = Wfull.shape[0]
    mdim = Wfull.shape[1]
    nrows = A_list[0].shape[0]
    P = 128
    kp = ((kdim + P - 1) // P) * P
    mh = mdim // 2
    mp = ((mh + P - 1) // P) * P
    npd = ((nrows + 511) // 512) * 512
    key = (kp, mp, npd)
    if key not in _DEV["cache"]:
        _DEV["cache"][key] = _build_mm(kp, mp, npd)
    nc = _DEV["cache"][key]
    W0 = _pad_to(Wfull[:, :mh], kp, mp)
    W1 = _pad_to(Wfull[:, mh:], kp, mp)
    ins = []
    for c in range(8):
        b = c % 4
        AT = _pad_to(np.ascontiguousarray(A_list[b].T), kp, npd)
        ins.append({"w": (W0 if c < 4 else W1), "a": AT})
    try:
        res = _DEV["run"](nc, ins, list(range(8)))
    except Exception as e:  # pragma: no cover
        print("kernel.py: device mm failed, numpy fallback:", e)
        _DEV["ok"] = False
        return [a.astype(np.float32) @ Wfull.astype(np.float32)
                for a in A_list]
    outs = []
    for b in range(4):
        y0 = res.results[b]["y"][:, :nrows]      # [mp, n] -> [mh, n]
        y1 = res.results[b + 4]["y"][:, :nrows]
        y = np.concatenate([y0[:mh], y1[:mh]], axis=0)   # [m, n]
        outs.append(np.ascontiguousarray(y.T))           # [n, m]
    return outs


# ---------------------------------------------------------------- host math
def _softmax(x, axis=-1):
    m = np.max(x, axis=axis, keepdims=True)
    e = np.exp(x - m)
    return e / np.sum(e, axis=axis, keepdims=True)


def _gelu(x):
    from scipy.special import erf
    return (x * 0.5 * (1.0 + erf(x / np.sqrt(np.float32(2.0))))).astype(
        np.float32)


def layer_norm(x, s, b):
    m = x.mean(-1, keepdims=True, dtype=np.float32)
    v = np.mean(np.square(x - m), -1, keepdims=True, dtype=np.float32)
    return ((x - m) / np.sqrt(v + 1e-5) * s + b).astype(np.float32)


def split_heads(t):
    b, n, _ = t.shape
    return t.reshape(b, n, HEADS, DH).transpose(0, 2, 1, 3)


def _qkv(x, Wqkv):
    xp = np.pad(x, ((0, 0), (0, 1), (0, 0)))
    qkv_rows = batched_mm([xp[i] for i in range(xp.shape[0])], Wqkv)
    qkv = np.stack(qkv_rows)                      # (B, NPAD, 3*DIM)
    q, k, v = np.split(qkv, 3, -1)
    return split_heads(q) * np.float32(SCALE), split_heads(k), split_heads(v)


def text_attn(qt, kt, vt):
    dots = np.einsum('bhid,bhjd->bhij', qt, kt).astype(np.float32)
    cm = np.triu(np.ones((TEXT_LEN, TEXT_LEN), bool), 1)
    dots = np.where(cm, NEG, dots)
    a = _softmax(dots, -1)
    return np.einsum('bhij,bhjd->bhid', a, vt).astype(np.float32)


def _split_ti(t):
    return t[:, :, :TEXT_LEN], t[:, :, TEXT_LEN:]


def _out_proj(out_heads, lp, n):
    # out_heads: (B, NPAD, DIM)
    rows = batched_mm([out_heads[i] for i in range(out_heads.shape[0])],
                      lp['Wo'])
    out = np.stack(rows) + lp['bo']
    return out[:, :n].astype(np.float32)


def axial_attn(x, lp, axis):
    b, n, _ = x.shape
    q, k, v = _qkv(x, lp['Wqkv'])
    qt, qi = _split_ti(q); kt, ki = _split_ti(k); vt, vi = _split_ti(v)
    out_t = text_attn(qt, kt, vt)

    def to_grid(t):
        t = t.reshape(b, HEADS, G, G, DH)
        return t if axis == 0 else t.transpose(0, 1, 3, 2, 4)
    qg, kg, vg = to_grid(qi), to_grid(ki), to_grid(vi)
    d_ii = np.einsum('bhxid,bhxjd->bhxij', qg, kg).astype(np.float32)
    cm = np.triu(np.ones((G, G), bool), 1)
    d_ii = np.where(cm, NEG, d_ii)
    d_it = np.einsum('bhxid,bhjd->bhxij', qg, kt).astype(np.float32)
    a = _softmax(np.concatenate([d_it, d_ii], -1), -1)
    a_it, a_ii = a[..., :TEXT_LEN], a[..., TEXT_LEN:]
    out_i = (np.einsum('bhxij,bhxjd->bhxid', a_ii, vg) +
             np.einsum('bhxij,bhjd->bhxid', a_it, vt)).astype(np.float32)
    if axis == 1:
        out_i = out_i.transpose(0, 1, 3, 2, 4)
    out_i = out_i.reshape(b, HEADS, IMG_LEN, DH)
    out = np.concatenate([out_t, out_i], 2)
    out = out.transpose(0, 2, 1, 3).reshape(b, n + 1, DIM)
    return _out_proj(out, lp, n)


def conv_attn(x, lp):
    b, n, _ = x.shape
    q, k, v = _qkv(x, lp['Wqkv'])
    qt, qi = _split_ti(q); kt, ki = _split_ti(k); vt, vi = _split_ti(v)
    out_t = text_attn(qt, kt, vt)

    def patches(t):
        tg = t.reshape(b, HEADS, G, G, DH)
        tp = np.pad(tg, ((0, 0), (0, 0), (PAD, PAD), (PAD, PAD), (0, 0)))
        return tp.reshape(b, HEADS, PG * PG, DH)[:, :, PIDX]
    kp, vp = patches(ki), patches(vi)
    d_ii = np.einsum('bhid,bhijd->bhij', qi, kp).astype(np.float32)
    d_ii = np.where(CONV_MASK, NEG, d_ii)
    d_it = np.einsum('bhid,bhjd->bhij', qi, kt).astype(np.float32)
    a = _softmax(np.concatenate([d_it, d_ii], -1), -1)
    a_it, a_ii = a[..., :TEXT_LEN], a[..., TEXT_LEN:]
    out_i = (np.einsum('bhij,bhijd->bhid', a_ii, vp) +
             np.einsum('bhij,bhjd->bhid', a_it, vt)).astype(np.float32)
    out = np.concatenate([out_t, out_i], 2)
    out = out.transpose(0, 2, 1, 3).reshape(b, n + 1, DIM)
    return _out_proj(out, lp, n)


def _to_np(x):
    if isinstance(x, dict):
        return {k: _to_np(v) for k, v in x.items()}
    if isinstance(x, (list, tuple)):
        return [_to_np(v) for v in x]
    a = np.asarray(x)
    if a.dtype == np.float64:
        a = a.astype(np.float32)
    return a


def kernel(audio, image, params):
    audio = np.asarray(audio, np.float32)
    image = np.asarray(image)
    p = _to_np(params)
    image_i = image.astype(np.int64)

    a = (audio @ p['W_in'] + p['b_in'] + p['audio_pos'][:AUDIO_LEN]).astype(
        np.float32)
    pos = (p['ax_row'][:, None, :] + p['ax_col'][None, :, :]).reshape(
        IMG_LEN, DIM)
    img = (p['img_emb'][image_i] + pos).astype(np.float32)
    x = np.concatenate([a, img], 1)              # (B, TOTAL, DIM)

    for lp, t in zip(p['layers'], ATTN_TYPES):
        h = layer_norm(x, lp['ln1_s'], lp['ln1_b'])
        if t == 'conv_like':
            ao = conv_attn(h, lp)
        else:
            ao = axial_attn(h, lp, 0 if t == 'axial_row' else 1)
        x = (x + ao * lp['g1']).astype(np.float32)
        h = layer_norm(x, lp['ln2_s'], lp['ln2_b'])
        h1_rows = batched_mm([h[i] for i in range(h.shape[0])], lp['W1'])
        h1 = np.stack(h1_rows) + lp['b1']
        u, g = np.split(h1.astype(np.float32), 2, -1)
        ug = (u * _gelu(g)).astype(np.float32)
        f_rows = batched_mm([ug[i] for i in range(ug.shape[0])], lp['W2'])
        f = np.stack(f_rows) + lp['b2']
        x = (x + f * lp['g2']).astype(np.float32)

    out = layer_norm(x[:, AUDIO_LEN:], p['ln_out_s'], p['ln_out_b'])
    lg_rows = batched_mm([out[i] for i in range(out.shape[0])], p['W_logit'])
    logits = np.stack(lg_rows) + p['b_logit']
    logits = logits.astype(np.float32)
    m = logits.max(-1, keepdims=True)
    lse = (m + np.log(np.sum(np.exp(logits - m), -1, keepdims=True))).astype(
        np.float32)
    logp = logits - lse
    tgt = np.take_along_axis(logp, image_i[..., None], -1)
    return np.float32(-tgt.mean())
